# revision 1
# baseline (speedup 1.0000x reference)
"""GraphSAGE 2-layer kernel for Trainium2, 8 NeuronCores, data-parallel over nodes.

Strategy:
- Nodes padded to 50176 = 8 cores * 49 tiles * 128; each core owns 6272 rows.
- One compiled Bass program = one SAGE layer for one core's slice:
    out[n, :] = relu(x[n] @ W_top + (mean_k x[nbr[n,k]]) @ W_bot + b)
  run twice (layer 1 on x, layer 2 on h1) with a host-side gather between.
- Per 128-node tile:
    * one indirect DMA gathers all 16 neighbor rows (2048 descriptors x 512B)
    * neighbor sum split between DVE (tensor_add) and PE (transpose-accumulate
      into PSUM via matmul with identity), mean folded into pre-scaled W_bot
    * self term uses a host-transposed slice of x so lhsT needs no on-chip
      transpose; two matmuls accumulate x@W_top + s@W_bot_scaled in PSUM
    * bias (host-broadcast to [128,128]) added on DVE, relu on ACT engine
"""

import sys

sys.path.insert(0, "/opt/trn_rl_repo")

from contextlib import ExitStack

import numpy as np

import concourse.bass as bass
import concourse.tile as tile
from concourse import mybir
from concourse.bass_utils import run_bass_kernel_spmd
from concourse.masks import make_identity

P = 128
DEG = 16
C = 8
N_DVE = 10  # neighbor blocks summed on DVE; rest transposed+summed on PE

_NC_CACHE = {}


def _split_wide_waits(nc, max_waits=1):
    """walrus codegen here allows a single sync-wait per instruction; move
    extra waits onto preceding nops on the same engine queue."""
    for fn in nc.m.functions:
        for bb in fn.blocks:
            out = []
            for inst in bb.instructions:
                si = inst.sync_info
                ow = list(si.on_wait) if si and si.on_wait else []
                limit = 0 if isinstance(inst, mybir.InstDrain) else max_waits
                if len(ow) > limit:
                    extra = ow if limit == 0 else ow[:-limit]
                    keep = [] if limit == 0 else ow[-limit:]
                    for k in range(0, len(extra), max_waits):
                        out.append(
                            mybir.InstNoOp(
                                name=f"{inst.name}-waitsplit{k}",
                                opcode="Nop",
                                engine=inst.engine,
                                debug=inst.debug,
                                ins=[],
                                outs=[],
                                sync_info=mybir.SyncInfo(
                                    on_wait=extra[k : k + max_waits], on_update=[]
                                ),
                                text_hint="waitsplit",
                                bass_nofuse=True,
                            )
                        )
                    si.on_wait = keep
                out.append(inst)
            bb.instructions[:] = out


def build_layer_nc(n_tiles, n_full):
    key = (n_tiles, n_full)
    if key in _NC_CACHE:
        return _NC_CACHE[key]
    f32 = mybir.dt.float32
    i32 = mybir.dt.int32
    nc = bass.Bass("TRN2", target_bir_lowering=False, debug=False, num_devices=C)
    xfull = nc.dram_tensor("xfull", [n_full, P], f32, kind="ExternalInput").ap()
    xt = nc.dram_tensor("xt", [P, n_tiles * P], f32, kind="ExternalInput").ap()
    nbr = nc.dram_tensor("nbr", [n_tiles * P, DEG], i32, kind="ExternalInput").ap()
    wt = nc.dram_tensor("wt", [P, P], f32, kind="ExternalInput").ap()
    wb = nc.dram_tensor("wb", [P, P], f32, kind="ExternalInput").ap()
    bbc = nc.dram_tensor("bbc", [P, P], f32, kind="ExternalInput").ap()
    out = nc.dram_tensor("out", [n_tiles * P, P], f32, kind="ExternalOutput").ap()

    with tile.TileContext(nc) as tc:
        with ExitStack() as ctx:
            consts = ctx.enter_context(tc.tile_pool(name="consts", bufs=1))
            gpool = ctx.enter_context(tc.tile_pool(name="gath", bufs=3))
            spool = ctx.enter_context(tc.tile_pool(name="sums", bufs=3))
            hpool = ctx.enter_context(tc.tile_pool(name="hh", bufs=3))
            xtp = ctx.enter_context(tc.tile_pool(name="xtp", bufs=3))
            nbrp = ctx.enter_context(tc.tile_pool(name="nbrp", bufs=3))
            ps_t = ctx.enter_context(tc.tile_pool(name="ps_t", bufs=2, space="PSUM"))
            ps_h = ctx.enter_context(tc.tile_pool(name="ps_h", bufs=2, space="PSUM"))

            ident = consts.tile([P, P], f32)
            make_identity(nc, ident[:])
            wt_sb = consts.tile([P, P], f32)
            nc.sync.dma_start(wt_sb[:], wt[:, :])
            wb_sb = consts.tile([P, P], f32)
            nc.sync.dma_start(wb_sb[:], wb[:, :])
            bb_sb = consts.tile([P, P], f32)
            nc.sync.dma_start(bb_sb[:], bbc[:, :])

            for t in range(n_tiles):
                nbr_t = nbrp.tile([P, DEG], i32)
                nc.sync.dma_start(nbr_t[:], nbr[t * P : (t + 1) * P, :])
                g = gpool.tile([P, DEG * P], f32)
                for k in range(DEG):
                    nc.gpsimd.indirect_dma_start(
                        out=g[:, k * P : (k + 1) * P],
                        out_offset=None,
                        in_=xfull[:, :],
                        in_offset=bass.IndirectOffsetOnAxis(
                            ap=nbr_t[:, k : k + 1], axis=0
                        ),
                    )
                acc = spool.tile([P, P], f32)
                nc.vector.tensor_add(acc[:], g[:, 0:P], g[:, P : 2 * P])
                for k in range(2, N_DVE):
                    nc.vector.tensor_add(acc[:], acc[:], g[:, k * P : (k + 1) * P])
                pst = ps_t.tile([P, P], f32)
                nc.tensor.matmul(
                    out=pst[:], lhsT=acc[:], rhs=ident[:], start=True, stop=False
                )
                for k in range(N_DVE, DEG):
                    nc.tensor.matmul(
                        out=pst[:],
                        lhsT=g[:, k * P : (k + 1) * P],
                        rhs=ident[:],
                        start=False,
                        stop=(k == DEG - 1),
                    )
                sT = spool.tile([P, P], f32)
                nc.scalar.copy(sT[:], pst[:])
                xt_t = xtp.tile([P, P], f32)
                nc.sync.dma_start(xt_t[:], xt[:, t * P : (t + 1) * P])
                psh = ps_h.tile([P, P], f32)
                nc.tensor.matmul(
                    out=psh[:], lhsT=xt_t[:], rhs=wt_sb[:], start=True, stop=False
                )
                nc.tensor.matmul(
                    out=psh[:], lhsT=sT[:], rhs=wb_sb[:], start=False, stop=True
                )
                hb = hpool.tile([P, P], f32)
                nc.vector.tensor_add(hb[:], psh[:], bb_sb[:])
                h = hpool.tile([P, P], f32)
                nc.scalar.activation(h[:], hb[:], mybir.ActivationFunctionType.Relu)
                nc.sync.dma_start(out[t * P : (t + 1) * P, :], h[:])

    _split_wide_waits(nc)
    _NC_CACHE[key] = nc
    return nc


def _run_layer(nc, xp, nbrp_arr, W, b, n_tiles, npc, trace=False):
    """xp: [n_full, P] f32 table; nbrp_arr: [n_full, DEG] i32. Returns [n_full, P]."""
    wt = np.ascontiguousarray(W[:P, :], dtype=np.float32)
    wb = np.ascontiguousarray(W[P:, :], dtype=np.float32) / np.float32(DEG)
    bbc = np.tile(np.asarray(b, dtype=np.float32).reshape(1, P), (P, 1))
    in_maps = []
    for c in range(C):
        sl = slice(c * npc, (c + 1) * npc)
        in_maps.append(
            {
                "xfull": xp,
                "xt": np.ascontiguousarray(xp[sl].T),
                "nbr": nbrp_arr[sl],
                "wt": wt,
                "wb": wb,
                "bbc": bbc,
            }
        )
    res = run_bass_kernel_spmd(nc, in_maps, core_ids=list(range(C)), trace=trace)
    h = np.concatenate([res.results[c]["out"] for c in range(C)], axis=0)
    return h, res


LAST_RUNS = []


def kernel(x, neighbors, W1, b1, W2, b2):
    N, D = x.shape
    assert D == P
    npc = -(-N // (C * P)) * P  # rows per core, padded to 128
    n_full = C * npc
    n_tiles = npc // P

    xp = np.zeros((n_full, P), dtype=np.float32)
    xp[:N] = np.asarray(x, dtype=np.float32)
    nbrp_arr = np.zeros((n_full, DEG), dtype=np.int32)
    nbrp_arr[:N] = np.asarray(neighbors).astype(np.int32)

    nc = build_layer_nc(n_tiles, n_full)
    h1, r1 = _run_layer(nc, xp, nbrp_arr, W1, b1, n_tiles, npc)
    out, r2 = _run_layer(nc, h1, nbrp_arr, W2, b2, n_tiles, npc)
    LAST_RUNS[:] = [r1, r2]
    return out[:N]

